# revision 5
# baseline (speedup 1.0000x reference)
"""Locally-connected layer (valid, 3x3, stride 1) on 8 Trainium2 NeuronCores.

Problem: X [128, 32, 32, 64], per-location filters [900, 576, 64]
         -> out [128, 30, 30, 64]   (out[n,l,f] = sum_k patch[n,l,k]*filt[l,k,f])

Sharding: each core owns 4 output rows (cores 6,7 overlap rows 26-27), so per
core: 6 input rows of X (6.3MB), 120 locations of filters (17.7MB), 3.9MB out
-> ~28MB of HBM traffic per core (memory-bound regime).

Kernel structure (per core):
 - X staged in SBUF as "paired-column" layout xs[(w%2)*64+c, h, w//2, n]: a
   [128,128] stationary tile holds two adjacent image columns (K=128).
 - Filters host-packed per even output-column group w0=2q into a 4-group
   layout [single(w0-2) | pair(w0-1) | pair(w0) | single(w0+1)] x 3 oh-slots
   so each (h, w0) patch tile needs only TWO wide matmuls (up to 2x3x64=384
   moving columns), with single-cell contributions zero-padded in the unused
   64 K-rows (zeros are memset once in SBUF, never DMA'd).
 - PSUM: one bank per ow-pair "rect" [128n, 2ow, 4oh, 64f] accumulates all 6
   contributions per location; retired via DVE copy + DMA as w0 advances.
"""

import numpy as np

import concourse.bass as bass
import concourse.mybir as mybir
import concourse.tile as tile
from concourse.bass_utils import run_bass_kernel_spmd

# ---------------- problem constants ----------------
N = 128
H = W = 32
FIN = FOUT = 64
OH = OW = 30
NR = 4                     # output rows per core
NHR = NR + 2               # input rows per core
R0S = [0, 4, 8, 12, 16, 20, 24, 26]   # per-core first output row
NQ = 16                    # w0 groups (w0 = 2q, q=0..15)
NRECT = 15                 # ow-pair rects per core

# per-h (slab) oh-slot ranges: slot j corresponds to oh = h-2+j
_JLO = [max(0, 2 - h) for h in range(6)]
_JHI = [min(3, 6 - h) for h in range(6)]
_NJ = [_JHI[h] - _JLO[h] for h in range(6)]          # 1,2,3,3,2,1
_OA = np.concatenate([[0], np.cumsum([2 * n for n in _NJ])]).astype(int)  # fp offsets
_OS = np.concatenate([[0], np.cumsum(_NJ)]).astype(int)                   # fsa/fsb offsets
FP_COLS = int(_OA[-1])     # 24
FS_COLS = int(_OS[-1])     # 12

_NC_CACHE = {}


def _max_sem_wait_fixup(nc):
    """This walrus build accepts at most one semaphore wait per instruction;
    move extra waits onto preceding same-engine NOPs (engine streams are
    in-order, so this is equivalent)."""
    for bb in nc.main_func.blocks:
        out = []
        for ins in bb.instructions:
            si = ins.sync_info
            if si is not None and si.on_wait and len(si.on_wait) > 1:
                extra, keep = si.on_wait[:-1], si.on_wait[-1:]
                for k, w in enumerate(extra):
                    out.append(mybir.InstNoOp(
                        name=f"{ins.name}-ws{k}",
                        engine=ins.engine,
                        sync_info=mybir.SyncInfo(on_wait=[w], on_update=[]),
                    ))
                si.on_wait = keep
            out.append(ins)
        bb.instructions[:] = out


def _build_nc():
    """Build the per-core Bass module (same NEFF on all 8 cores)."""
    f32 = mybir.dt.float32
    nc = bass.Bass()

    xs_d = nc.dram_tensor("xs", [128, 6, NQ, N], f32, kind="ExternalInput")
    fp_d = nc.dram_tensor("fp", [128, NQ, FP_COLS, FOUT], f32, kind="ExternalInput")
    fsa_d = nc.dram_tensor("fsa", [64, NQ, FS_COLS, FOUT], f32, kind="ExternalInput")
    fsb_d = nc.dram_tensor("fsb", [64, NQ, FS_COLS, FOUT], f32, kind="ExternalInput")
    ot_d = nc.dram_tensor("ot", [N, NRECT, 512], f32, kind="ExternalOutput")

    with tile.TileContext(nc) as tc:
        with (
            tc.tile_pool(name="xpool", bufs=1) as xpool,
            tc.tile_pool(name="fpool", bufs=1) as fpool,
            tc.tile_pool(name="opool", bufs=3) as opool,
            tc.tile_pool(name="psum", bufs=4, space="PSUM") as pspool,
        ):
            # X slabs, loaded once
            xs_t = []
            for h in range(6):
                t = xpool.tile([128, NQ, N], f32, tag=f"xs{h}", name=f"xs{h}")
                nc.sync.dma_start(t[:], xs_d[:, h, :, :])
                xs_t.append(t)

            # Double-buffered filter tiles [128, h, grp, ohslot, f]
            ft = [fpool.tile([128, 6, 4, 3, FOUT], f32, tag=f"ft{b}", name=f"ft{b}")
                  for b in range(2)]
            # zero the K-halves that single-cell contributions must not touch
            for b in range(2):
                nc.vector.memset(ft[b][64:128, :, 0, :, :], 0.0)
                nc.vector.memset(ft[b][0:64, :, 3, :, :], 0.0)

            rects = [None] * NRECT

            def retire(p):
                stg = opool.tile([128, 512], f32, tag="stg", name="stg")
                nc.vector.tensor_copy(stg[:], rects[p].rearrange("p a b f -> p (a b f)"))
                nc.sync.dma_start(ot_d[:, p, :], stg[:])

            for q in range(NQ):
                f = ft[q % 2]
                for h in range(6):
                    jlo, jhi, nj = _JLO[h], _JHI[h], _NJ[h]
                    nc.sync.dma_start(
                        f[:, h, 1:3, jlo:jhi, :],
                        fp_d[:, q, int(_OA[h]):int(_OA[h]) + 2 * nj, :]
                        .rearrange("p (g j) f -> p g j f", g=2),
                    )
                    if q >= 1:
                        nc.sync.dma_start(
                            f[0:64, h, 0, jlo:jhi, :],
                            fsa_d[:, q, int(_OS[h]):int(_OS[h]) + nj, :],
                        )
                    if q <= NQ - 2:
                        nc.sync.dma_start(
                            f[64:128, h, 3, jlo:jhi, :],
                            fsb_d[:, q, int(_OS[h]):int(_OS[h]) + nj, :],
                        )
                if q <= NQ - 2:
                    rects[q] = pspool.tile([128, 2, 4, FOUT], f32, name=f"rect{q}",
                                           tag="rect")
                for h in range(6):
                    jlo, jhi, nj = _JLO[h], _JHI[h], _NJ[h]
                    ohlo = h - 2 + jlo
                    lhsT = xs_t[h][:, q, :]
                    if q >= 1:
                        nc.tensor.matmul(
                            rects[q - 1][:, :, ohlo:ohlo + nj, :],
                            lhsT,
                            f[:, h, 0:2, jlo:jhi, :],
                            start=False,
                            stop=(h == 5),
                        )
                    if q <= NQ - 2:
                        nc.tensor.matmul(
                            rects[q][:, :, ohlo:ohlo + nj, :],
                            lhsT,
                            f[:, h, 2:4, jlo:jhi, :],
                            start=(h == 0),
                            stop=False,
                        )
                if q >= 1:
                    retire(q - 1)

    _max_sem_wait_fixup(nc)
    return nc


def _get_nc():
    if "nc" not in _NC_CACHE:
        _NC_CACHE["nc"] = _build_nc()
    return _NC_CACHE["nc"]


def _host_pack_core(X, filters, R0):
    """Build the per-core input arrays."""
    # xs[(w%2)*64+c, hr, w//2, n] = X[n, R0+hr, w, c]
    Xc = X[:, R0:R0 + 6]                                # [n, 6, 32, c]
    xs = np.ascontiguousarray(
        Xc.reshape(N, 6, NQ, 2, FIN).transpose(3, 4, 1, 2, 0).reshape(128, 6, NQ, N)
    )

    fk = filters.reshape(OH * OW, 9, 64, FOUT)          # [l, (fh,fw), c, f]
    fp = np.zeros((128, NQ, FP_COLS, FOUT), np.float32)
    fsa = np.zeros((64, NQ, FS_COLS, FOUT), np.float32)
    fsb = np.zeros((64, NQ, FS_COLS, FOUT), np.float32)
    for q in range(NQ):
        w0 = 2 * q
        for h in range(6):
            jlo, jhi, nj = _JLO[h], _JHI[h], _NJ[h]
            for jj in range(jlo, jhi):
                oh = h - 2 + jj
                fh = 2 - jj
                # pairs: g=0 -> ow=w0-1 (fw 1,2); g=1 -> ow=w0 (fw 0,1)
                for g in range(2):
                    ow = w0 - 1 + g
                    if not (0 <= ow < OW):
                        continue
                    l = (R0 + oh) * OW + ow
                    col = int(_OA[h]) + g * nj + (jj - jlo)
                    fw = w0 - ow
                    fp[0:64, q, col, :] = fk[l, 3 * fh + fw]
                    fp[64:128, q, col, :] = fk[l, 3 * fh + fw + 1]
                # single A: ow = w0-2 (cell w0, fw=2), K-rows 0:64
                ow = w0 - 2
                if 0 <= ow < OW:
                    l = (R0 + oh) * OW + ow
                    fsa[:, q, int(_OS[h]) + (jj - jlo), :] = fk[l, 3 * fh + 2]
                # single B: ow = w0+1 (cell w0+1, fw=0), K-rows 64:128
                ow = w0 + 1
                if 0 <= ow < OW:
                    l = (R0 + oh) * OW + ow
                    fsb[:, q, int(_OS[h]) + (jj - jlo), :] = fk[l, 3 * fh + 0]
    return {"xs": xs, "fp": fp, "fsa": fsa, "fsb": fsb}


def kernel(X, filters):
    X = np.asarray(X, dtype=np.float32)
    filters = np.asarray(filters, dtype=np.float32)
    nc = _get_nc()
    in_maps = [_host_pack_core(X, filters, R0) for R0 in R0S]
    res = run_bass_kernel_spmd(nc, in_maps, core_ids=list(range(8)), trace=False)

    out = np.empty((N, OH, OW, FOUT), np.float32)
    for i, R0 in enumerate(R0S):
        ot = res.results[i]["ot"]                        # [n, 15, 512]
        core = (
            ot.reshape(N, NRECT, 2, 4, FOUT)
            .transpose(0, 3, 1, 2, 4)
            .reshape(N, 4, OW, FOUT)
        )
        out[:, R0:R0 + 4] = core
    return out


# revision 12
# speedup vs baseline: 23429.5513x; 23429.5513x over previous
"""Locally-connected layer (valid, 3x3, stride 1) on 8 Trainium2 NeuronCores.

Problem: X [128, 32, 32, 64], per-location filters [900, 576, 64]
         -> out [128, 30, 30, 64]   (out[n,l,f] = sum_k patch[n,l,k]*filt[l,k,f])

Sharding: each core owns 4 output rows (cores 6,7 overlap rows 26-27), so per
core: 6 input rows of X (6.3MB), 120 locations of filters (17.7MB), 3.9MB out
-> ~28MB of HBM traffic per core (memory-bound regime).

Kernel structure (per core):
 - X staged in SBUF as "paired-column" layout xs[(w%2)*64+c, h, w//2, n]: a
   [128,128] stationary (patch) tile holds two adjacent image columns (K=128);
   matmuls are patch-stationary / filter-moving: out[n, (ow,oh,f)] in PSUM.
 - Filters host-packed per even output-column group w0=2q (pairs + two
   single-cell tensors), fetched as 3 large contiguous DMAs per q, spread
   over both HWDGE engines (SP + ACT).
 - Per (q, h): 4 fp32 matmuls — two K=128 pair-contributions (ow=w0-1, w0)
   and two K=64 single-cell contributions (ow=w0-2 on partitions 0:64,
   ow=w0+1 on partitions 64:128), each writing up to [128, 3oh, 64f].
 - PSUM: one bank per ow-pair "rect" [128n, 2ow, 4oh, 64f] accumulates all 6
   contributions per location; retired via DVE copy + DMA as w0 advances.
"""

import numpy as np

import concourse.bass as bass
import concourse.mybir as mybir
import concourse.tile as tile
from concourse.bass_utils import run_bass_kernel_spmd

# ---------------- problem constants ----------------
N = 128
H = W = 32
FIN = FOUT = 64
OH = OW = 30
NR = 4                     # output rows per core
NHR = NR + 2               # input rows per core
R0S = [0, 4, 8, 12, 16, 20, 24, 26]   # per-core first output row
NQ = 16                    # w0 groups (w0 = 2q, q=0..15)
NRECT = 15                 # ow-pair rects per core

# per-h (slab) oh-slot ranges: slot j corresponds to oh = h-2+j
_JLO = [max(0, 2 - h) for h in range(6)]
_JHI = [min(3, 6 - h) for h in range(6)]
_NJ = [_JHI[h] - _JLO[h] for h in range(6)]          # 1,2,3,3,2,1
_OA = np.concatenate([[0], np.cumsum([2 * n for n in _NJ])]).astype(int)  # fp offsets
_OS = np.concatenate([[0], np.cumsum(_NJ)]).astype(int)                   # fsa/fsb offsets
FP_COLS = int(_OA[-1])     # 24
FS_COLS = int(_OS[-1])     # 12

_NC_CACHE = {}


def _max_sem_wait_fixup(nc):
    """This walrus build accepts at most one semaphore wait per instruction;
    move extra waits onto preceding same-engine NOPs (engine streams are
    in-order, so this is equivalent)."""
    for bb in nc.main_func.blocks:
        out = []
        for ins in bb.instructions:
            si = ins.sync_info
            if si is not None and si.on_wait and len(si.on_wait) > 1:
                extra, keep = si.on_wait[:-1], si.on_wait[-1:]
                for k, w in enumerate(extra):
                    out.append(mybir.InstNoOp(
                        name=f"{ins.name}-ws{k}",
                        engine=ins.engine,
                        sync_info=mybir.SyncInfo(on_wait=[w], on_update=[]),
                    ))
                si.on_wait = keep
            out.append(ins)
        bb.instructions[:] = out


def _build_nc(reps=1, do_dma=True, do_mm=True):
    """Build the per-core Bass module (same NEFF on all 8 cores).

    reps>1 wraps the body in an on-device For_i loop (timing only);
    do_dma/do_mm selectively disable stages (bottleneck diagnosis only).
    """
    from contextlib import nullcontext

    f32 = mybir.dt.float32
    nc = bass.Bass()

    xs_d = nc.dram_tensor("xs", [128, 6, NQ, N], f32, kind="ExternalInput")
    fp_d = nc.dram_tensor("fp", [128, NQ, FP_COLS, FOUT], f32, kind="ExternalInput")
    fsa_d = nc.dram_tensor("fsa", [64, NQ, FS_COLS, FOUT], f32, kind="ExternalInput")
    fsb_d = nc.dram_tensor("fsb", [64, NQ, FS_COLS, FOUT], f32, kind="ExternalInput")
    ot_d = nc.dram_tensor("ot", [N, NRECT, 512], f32, kind="ExternalOutput")

    with tile.TileContext(nc) as tc:
        with (
            tc.tile_pool(name="xpool", bufs=1) as xpool,
            tc.tile_pool(name="fpool", bufs=3) as fpool,
            tc.tile_pool(name="opool", bufs=3) as opool,
            tc.tile_pool(name="psum", bufs=4, space="PSUM") as pspool,
        ):
            # X slabs, loaded once (split across both HWDGE engines)
            xs_t = []
            for h in range(6):
                t = xpool.tile([128, NQ, N], f32, tag=f"xs{h}", name=f"xs{h}")
                if do_dma:
                    eng = nc.sync if h % 2 == 0 else nc.scalar
                    eng.dma_start(t[:], xs_d[:, h, :, :])
                xs_t.append(t)

            loop = tc.For_i(0, reps, 1) if reps > 1 else nullcontext()
            with loop:
                rects = [None] * NRECT

                def retire(p):
                    stg = opool.tile([128, 512], f32, tag="stg", name="stg")
                    nc.vector.tensor_copy(
                        stg[:], rects[p].rearrange("p a b f -> p (a b f)"))
                    nc.scalar.dma_start(ot_d[:, p, :], stg[:])

                for q in range(NQ):
                    # one fat DMA per filter tensor per q (packed layouts)
                    ftp = fpool.tile([128, FP_COLS, FOUT], f32, tag="ftp", name="ftp")
                    fta = fpool.tile([128, FS_COLS, FOUT], f32, tag="fta", name="fta")
                    ftb = fpool.tile([128, FS_COLS, FOUT], f32, tag="ftb", name="ftb")
                    if do_dma:
                        nc.sync.dma_start(ftp[:], fp_d[:, q, :, :])
                        if q >= 1:
                            nc.scalar.dma_start(fta[0:64], fsa_d[:, q, :, :])
                        if q <= NQ - 2:
                            nc.scalar.dma_start(ftb[64:128], fsb_d[:, q, :, :])
                    if not do_mm:
                        continue
                    if q <= NQ - 2:
                        rects[q] = pspool.tile([128, 2, 4, FOUT], f32,
                                               name=f"rect{q}", tag="rect")
                    for h in range(6):
                        jlo, nj = _JLO[h], _NJ[h]
                        oA, oS = int(_OA[h]), int(_OS[h])
                        ohlo = h - 2 + jlo
                        osl = slice(ohlo, ohlo + nj)
                        if q >= 1:
                            # pair member ow=w0-1 (K=128)
                            nc.tensor.matmul(
                                rects[q - 1][:, 1, osl, :],
                                xs_t[h][:, q, :],
                                ftp[:, oA:oA + nj, :],
                                start=False, stop=False,
                            )
                            # single ow=w0-2, cell w0 (K=64, rows 0:64)
                            nc.tensor.matmul(
                                rects[q - 1][:, 0, osl, :],
                                xs_t[h][0:64, q, :],
                                fta[0:64, oS:oS + nj, :],
                                start=False, stop=(h == 5),
                            )
                        if q <= NQ - 2:
                            # pair member ow=w0 (K=128)
                            nc.tensor.matmul(
                                rects[q][:, 0, osl, :],
                                xs_t[h][:, q, :],
                                ftp[:, oA + nj:oA + 2 * nj, :],
                                start=(h == 0), stop=False,
                            )
                            # single ow=w0+1, cell w0+1 (K=64, rows 64:128)
                            nc.tensor.matmul(
                                rects[q][:, 1, osl, :],
                                xs_t[h][64:128, q, :],
                                ftb[64:128, oS:oS + nj, :],
                                start=False, stop=False,
                            )
                    if q >= 1:
                        retire(q - 1)

    _max_sem_wait_fixup(nc)
    return nc


def _get_nc():
    if "nc" not in _NC_CACHE:
        _NC_CACHE["nc"] = _build_nc()
    return _NC_CACHE["nc"]


def _host_pack_core(X, filters, R0):
    """Build the per-core input arrays."""
    # xs[(w%2)*64+c, hr, w//2, n] = X[n, R0+hr, w, c]
    Xc = X[:, R0:R0 + 6]                                # [n, 6, 32, c]
    xs = np.ascontiguousarray(
        Xc.reshape(N, 6, NQ, 2, FIN).transpose(3, 4, 1, 2, 0).reshape(128, 6, NQ, N)
    )

    fk = filters.reshape(OH * OW, 9, 64, FOUT)          # [l, (fh,fw), c, f]
    fp = np.zeros((128, NQ, FP_COLS, FOUT), np.float32)
    fsa = np.zeros((64, NQ, FS_COLS, FOUT), np.float32)
    fsb = np.zeros((64, NQ, FS_COLS, FOUT), np.float32)
    for q in range(NQ):
        w0 = 2 * q
        for h in range(6):
            jlo, jhi, nj = _JLO[h], _JHI[h], _NJ[h]
            for jj in range(jlo, jhi):
                oh = h - 2 + jj
                fh = 2 - jj
                # pairs: g=0 -> ow=w0-1 (fw 1,2); g=1 -> ow=w0 (fw 0,1)
                for g in range(2):
                    ow = w0 - 1 + g
                    if not (0 <= ow < OW):
                        continue
                    l = (R0 + oh) * OW + ow
                    col = int(_OA[h]) + g * nj + (jj - jlo)
                    fw = w0 - ow
                    fp[0:64, q, col, :] = fk[l, 3 * fh + fw]
                    fp[64:128, q, col, :] = fk[l, 3 * fh + fw + 1]
                # single A: ow = w0-2 (cell w0, fw=2), K-rows 0:64
                ow = w0 - 2
                if 0 <= ow < OW:
                    l = (R0 + oh) * OW + ow
                    fsa[:, q, int(_OS[h]) + (jj - jlo), :] = fk[l, 3 * fh + 2]
                # single B: ow = w0+1 (cell w0+1, fw=0), K-rows 64:128
                ow = w0 + 1
                if 0 <= ow < OW:
                    l = (R0 + oh) * OW + ow
                    fsb[:, q, int(_OS[h]) + (jj - jlo), :] = fk[l, 3 * fh + 0]
    return {"xs": xs, "fp": fp, "fsa": fsa, "fsb": fsb}


def kernel(X, filters):
    X = np.asarray(X, dtype=np.float32)
    filters = np.asarray(filters, dtype=np.float32)
    nc = _get_nc()
    in_maps = [_host_pack_core(X, filters, R0) for R0 in R0S]
    res = run_bass_kernel_spmd(nc, in_maps, core_ids=list(range(8)), trace=False)

    out = np.empty((N, OH, OW, FOUT), np.float32)
    for i, R0 in enumerate(R0S):
        ot = res.results[i]["ot"]                        # [n, 15, 512]
        core = (
            ot.reshape(N, NRECT, 2, 4, FOUT)
            .transpose(0, 3, 1, 2, 4)
            .reshape(N, 4, OW, FOUT)
        )
        out[:, R0:R0 + 4] = core
    return out
